# revision 58
# baseline (speedup 1.0000x reference)
"""Trainium2 Bass kernel for nn_GraphSemanticExtractor (GNN message passing).

Sharding (8 NeuronCores), 5 launches with host-side layout glue between them:
  P0: core c => 128-row chunk of M = phi_w @ psi_w.T, plus V = W^T [a_src|a_dst]
      for (layer l=c//4, head hd=c%4).
  P1 (edge build): core c => (batch b=c//4, 256-row chunk rc=c%4);
      scores = (x_c @ M) @ x.T, top-8, softmax over the 8, self-edge mask.
  P2/P3 (GAT layers 1/2): core c => (batch b=c//4, head hd=c%4); between the
      two, the host computes x1 = relu(sum of per-head partials).
  P4: pool + projection head, core b in {0,1}.

The sparse top-k aggregation out[dst] += wgt*h[src] is a dense matmul
out.T = h.T @ R with R[s,t] = ew_k(s)*exp(lrelu(e_src[s]+e_dst[t])) at
t=topi[s,k]; R is built by scattering ew into M0 (gpsimd local_scatter) and a
dense lrelu/exp of the rank-1 e-grid, all overlapped with the h matmul on PE.
"""

import sys

sys.path.insert(0, "/opt/trn_rl_repo")
sys.path.insert(0, "/opt/trn_rl_repo/concourse")

from contextlib import ExitStack

import ml_dtypes
import numpy as np

import concourse.bass as bass
import concourse.tile as tile
from concourse import bacc, mybir
from concourse.bass_utils import run_bass_kernel_spmd

F32 = mybir.dt.float32
BF16 = mybir.dt.bfloat16
F8 = mybir.dt.float8e4
U32 = mybir.dt.uint32
I16 = mybir.dt.int16
AF = mybir.ActivationFunctionType
OP = mybir.AluOpType
AX = mybir.AxisListType
DR = mybir.MatmulPerfMode.DoubleRow

B, S, H = 2, 1024, 1024
HEADS, K = 4, 8
SEM = 512
NB = H // 128  # 8 partition blocks
ND = NB // 2   # 4 double-row blocks for fp8 DoubleRow matmuls
CH = S // 4    # 256 rows per edge-build core

# fp8 e4m3 has min-normal 2^-6; the tiny GAT weights (~0.02 scale) are scaled
# up on the host and the factors folded back into on-device scalars.
W_SCALE = 32.0   # W.T fed to the h matmul
V_SCALE = 64.0   # V = W^T [a_src|a_dst] fed to the e matmuls
A_SCALE = 64.0   # gat_att halves fed to P0's V matmul


def _build_P0(nc):
    """Per core: 128 rows of M = phi_w.T @ psi_w, and V = W^T [a_src|a_dst]
    for one (layer, head).  The V matmul runs in fp8 DoubleRow (host scales
    its inputs by A_SCALE*W_SCALE; the eviction scales back)."""
    pTc = nc.dram_tensor("pTc", [H, 128], BF16, kind="ExternalInput")
    sT = nc.dram_tensor("sT", [H, H], BF16, kind="ExternalInput")
    Wn = nc.dram_tensor("Wn", [H, H], F8, kind="ExternalInput")
    a2 = nc.dram_tensor("a2", [H, 2], F8, kind="ExternalInput")
    Mc = nc.dram_tensor("Mc", [128, H], BF16, kind="ExternalOutput")
    VT = nc.dram_tensor("VT", [2, H], F32, kind="ExternalOutput")

    with tile.TileContext(nc) as tc, ExitStack() as ctx:
        pers = ctx.enter_context(tc.tile_pool(name="pers", bufs=1))
        psum = ctx.enter_context(tc.tile_pool(name="psum", bufs=4, space="PSUM"))

        # all input DMAs on the sync queue, smallest first (single DMA engine
        # processes FIFO; a big transfer queued first would stall the rest)
        a2t = pers.tile([128, NB, 2], F8, tag="a2t")
        nc.sync.dma_start(out=a2t[:], in_=a2[:].rearrange("(kb p) c -> p kb c", p=128))
        pT16 = pers.tile([128, NB, 128], BF16, tag="pT16")
        nc.sync.dma_start(out=pT16[:], in_=pTc[:].rearrange("(kb p) c -> p kb c", p=128))
        Wn16 = pers.tile([128, NB, H], F8, tag="Wn16")
        nc.sync.dma_start(out=Wn16[:], in_=Wn[:].rearrange("(kb p) s -> p kb s", p=128))
        sT16 = pers.tile([128, NB, H], BF16, tag="sT16")
        nc.sync.dma_start(out=sT16[:], in_=sT[:].rearrange("(kb p) s -> p kb s", p=128))

        Vt = pers.tile([2, H], F32, tag="Vt")
        ptV = [psum.tile([2, 512], F32, tag="vm", name=f"vm{i}") for i in range(2)]
        for k in range(NB):
            for i in range(2):
                nc.tensor.matmul(ptV[i][:], a2t[:, k, :], Wn16[:, k, 512 * i:512 * i + 512],
                                 start=(k == 0), stop=(k == NB - 1))
        for i in range(2):
            nc.vector.tensor_scalar(Vt[:, 512 * i:512 * i + 512], ptV[i][:],
                                    1.0 / (A_SCALE * W_SCALE), None, op0=OP.mult)
        nc.sync.dma_start(out=VT[:], in_=Vt[:])

        Mc16 = pers.tile([128, H], BF16, tag="Mc16")
        ptM = [psum.tile([128, 512], F32, tag="mm", name=f"mm{i}") for i in range(2)]
        for k in range(NB):
            for i in range(2):
                nc.tensor.matmul(ptM[i][:], pT16[:, k, :], sT16[:, k, 512 * i:512 * i + 512],
                                 start=(k == 0), stop=(k == NB - 1))
        for i in range(2):
            nc.vector.tensor_copy(out=Mc16[:, 512 * i:512 * i + 512], in_=ptM[i][:])
        nc.sync.dma_start(out=Mc[:], in_=Mc16[:])
    nc.compile()
    return nc


def _build_P1(nc):
    """Edge build: scores = (x_c @ M) @ x.T, top-8 + softmax + self-mask."""
    xT = nc.dram_tensor("xT", [H, S], BF16, kind="ExternalInput")
    xTc = nc.dram_tensor("xTc", [H, CH], BF16, kind="ExternalInput")
    Mm = nc.dram_tensor("Mm", [H, H], BF16, kind="ExternalInput")
    srcx = nc.dram_tensor("srcx", [CH, 1], F32, kind="ExternalInput")
    topi = nc.dram_tensor("topi", [CH, K], U32, kind="ExternalOutput")
    ew = nc.dram_tensor("ew", [CH, K], F32, kind="ExternalOutput")

    with tile.TileContext(nc) as tc, ExitStack() as ctx:
        pers = ctx.enter_context(tc.tile_pool(name="pers", bufs=1))
        psum = ctx.enter_context(tc.tile_pool(name="psum", bufs=4, space="PSUM"))
        psumb = ctx.enter_context(tc.tile_pool(name="psumb", bufs=4, space="PSUM"))

        sx = pers.tile([128, 2, 1], F32, tag="sx")
        nc.sync.dma_start(out=sx[:], in_=srcx[:].rearrange("(m p) c -> p m c", p=128))
        xTc16 = pers.tile([128, NB, CH], BF16, tag="xTc16")
        nc.sync.dma_start(out=xTc16[:], in_=xTc[:].rearrange("(kb p) s -> p kb s", p=128))
        M16 = pers.tile([128, NB, H], BF16, tag="M16")
        nc.sync.dma_start(out=M16[:], in_=Mm[:].rearrange("(kb p) s -> p kb s", p=128))
        xT16 = pers.tile([128, NB, S], BF16, tag="xT16")
        nc.sync.dma_start(out=xT16[:], in_=xT[:].rearrange("(kb p) s -> p kb s", p=128))

        # preload the Exp act table while DMAs run so the top-k chain's exp
        # doesn't pay the 1.3us table load; warm the PE p-state too
        warm = pers.tile([1, 1], F32, tag="warm")
        nc.vector.memset(warm[:], 0.0)
        nc.scalar.activation(warm[:], warm[:], AF.Exp)

        # PT[j, s-chunk] = (x_c @ M).T  (j = feature of M's column space)
        PT16 = pers.tile([128, NB, CH], BF16, tag="PT16")
        for m in range(NB):
            pt = psumb.tile([128, CH], F32, tag="ptm")
            for k in range(NB):
                nc.tensor.matmul(pt[:], M16[:, k, m * 128:(m + 1) * 128], xTc16[:, k, :],
                                 start=(k == 0), stop=(k == NB - 1))
            nc.vector.tensor_copy(out=PT16[:, m, :], in_=pt[:])

        # scores [s-chunk, t] f32; the whole per-sb top-8/softmax/mask chain runs
        # while the other sb-block's matmuls occupy PE
        sc = pers.tile([128, 2, S], F32, tag="scores")
        mv = pers.tile([128, 2, K], F32, tag="mv")
        ti = pers.tile([128, 2, K], U32, tag="ti")
        ex = pers.tile([128, 2, K], F32, tag="ex")
        sm = pers.tile([128, 2, 1], F32, tag="sm")
        rc = pers.tile([128, 2, 1], F32, tag="rc")
        tif = pers.tile([128, 2, K], F32, tag="tif")
        w8 = pers.tile([128, 2, K], F32, tag="w8")
        msk = pers.tile([128, 2, K], F32, tag="msk")
        ewt = pers.tile([128, 2, K], F32, tag="ewt")
        topir = topi[:].rearrange("(m p) k -> p m k", p=128)
        ewr = ew[:].rearrange("(m p) k -> p m k", p=128)
        for sb in range(2):
            for n0 in range(0, S, 512):
                pt = psum.tile([128, 512], F32, tag="scm")
                for k in range(NB):
                    nc.tensor.matmul(pt[:], PT16[:, k, sb * 128:(sb + 1) * 128],
                                     xT16[:, k, n0:n0 + 512],
                                     start=(k == 0), stop=(k == NB - 1))
                nc.vector.tensor_copy(out=sc[:, sb, n0:n0 + 512], in_=pt[:])
            nc.vector.max(mv[:, sb, :], sc[:, sb, :])
            nc.vector.max_index(ti[:, sb, :], mv[:, sb, :], sc[:, sb, :])
            nc.sync.dma_start(out=topir[:, sb, :], in_=ti[:, sb, :])
            nc.scalar.activation(ex[:, sb, :], mv[:, sb, :], AF.Exp)
            nc.vector.tensor_reduce(sm[:, sb, :], ex[:, sb, :], axis=AX.X, op=OP.add)
            nc.vector.tensor_scalar(sm[:, sb, :], sm[:, sb, :], 1e-8, None, op0=OP.add)
            nc.vector.reciprocal(rc[:, sb, :], sm[:, sb, :])
            nc.vector.tensor_copy(out=tif[:, sb, :], in_=ti[:, sb, :])
            nc.vector.tensor_scalar(w8[:, sb, :], ex[:, sb, :], rc[:, sb, :], 1e-8, op0=OP.mult, op1=OP.max)
            nc.vector.tensor_scalar(msk[:, sb, :], tif[:, sb, :], sx[:, sb, :], None, op0=OP.is_equal)
            nc.vector.tensor_scalar(msk[:, sb, :], msk[:, sb, :], -1.0, 1.0, op0=OP.mult, op1=OP.add)
            nc.vector.tensor_tensor(ewt[:, sb, :], w8[:, sb, :], msk[:, sb, :], op=OP.mult)
            nc.sync.dma_start(out=ewr[:, sb, :], in_=ewt[:, sb, :])
    nc.compile()
    return nc


def _build_L(nc):
    """One GAT layer for one (batch, head).  gT[feat, node] = (agg/attn)/HEADS.
    The h matmul runs fp8 DoubleRow with W split into an fp8 hi/lo residual
    pair (hi + lo/16 restores ~bf16 weight accuracy; fp8 W alone costs 2e-2
    output error).  x and V are plain fp8 (~2e-3 each).  R and the
    aggregation stay bf16: fp8 R alone costs 4e-2."""
    xT = nc.dram_tensor("xT", [H, S], F8, kind="ExternalInput")
    WTh = nc.dram_tensor("WTh", [H, H], F8, kind="ExternalInput")
    WTl = nc.dram_tensor("WTl", [H, H], F8, kind="ExternalInput")
    V2 = nc.dram_tensor("V2", [H, 2], F8, kind="ExternalInput")
    tpi = nc.dram_tensor("tpi", [S, K], I16, kind="ExternalInput")
    ewd = nc.dram_tensor("ewd", [S, K], BF16, kind="ExternalInput")
    sl2 = nc.dram_tensor("sl2", [2, 128], BF16, kind="ExternalInput")
    gT = nc.dram_tensor("gT", [H, S], BF16, kind="ExternalOutput")
    atO = nc.dram_tensor("atO", [1, S], F32, kind="ExternalOutput")

    with tile.TileContext(nc) as tc, ExitStack() as ctx:
        pers = ctx.enter_context(tc.tile_pool(name="pers", bufs=1))
        psum = ctx.enter_context(tc.tile_pool(name="psum", bufs=2, space="PSUM"))
        pse = ctx.enter_context(tc.tile_pool(name="pse", bufs=1, space="PSUM"))
        psx = ctx.enter_context(tc.tile_pool(name="psx", bufs=1, space="PSUM"))

        # all inputs on the sync queue, smallest first (FIFO DMA engine)
        sel = pers.tile([2, 128], BF16, tag="sel")
        nc.sync.dma_start(out=sel[:], in_=sl2[:])
        V16 = pers.tile([128, NB, 2], F8, tag="V16")
        nc.sync.dma_start(out=V16[:], in_=V2[:].rearrange("(kb p) c -> p kb c", p=128))
        tpw = pers.tile([128, NB, K], I16, tag="tpw")
        nc.sync.dma_start(out=tpw[:], in_=tpi[:].rearrange("(m p) k -> p m k", p=128))
        ews16 = pers.tile([128, NB, K], BF16, tag="ews16")
        nc.sync.dma_start(out=ews16[:], in_=ewd[:].rearrange("(m p) k -> p m k", p=128))
        xT16 = pers.tile([128, NB, S], F8, tag="xT16")  # noqa: dma order: tiny first
        nc.sync.dma_start(out=xT16[:], in_=xT[:].rearrange("(kb p) s -> p kb s", p=128))
        WTh16 = pers.tile([128, NB, H], F8, tag="WTh16")
        nc.sync.dma_start(out=WTh16[:], in_=WTh[:].rearrange("(kb p) s -> p kb s", p=128))
        WTl16 = pers.tile([128, NB, H], F8, tag="WTl16")
        nc.sync.dma_start(out=WTl16[:], in_=WTl[:].rearrange("(kb p) s -> p kb s", p=128))

        ones11 = pers.tile([1, 1], BF16, tag="ones11")
        nc.vector.memset(ones11[:], 1.0)
        # preload the Lrelu act table while DMAs run
        warm = pers.tile([1, 1], F32, tag="warm")
        nc.vector.memset(warm[:], 0.0)
        nc.scalar.activation(warm[:], warm[:], AF.Lrelu, alpha=0.2)

        # gpsimd: M0 blocks (scatter of ew into dense [s, t])
        M0 = pers.tile([128, NB, S], BF16, tag="M0")
        for m in range(NB):
            nc.gpsimd.local_scatter(M0[:, m, :], ews16[:, m, :], tpw[:, m, :],
                                    channels=128, num_elems=S, num_idxs=K)

        # PE: e_bothT [2, node] = V^T x as two independent k-split psum groups
        # (independent groups queue up and run at ramped PE speed); eb2 keeps
        # the V_SCALE factor — the exp pass divides it out via its scale param
        # (lrelu is positively homogeneous so the order is exact)
        eb2 = pers.tile([2, S], BF16, tag="eb2")
        tmpe = pers.tile([2, 2, 512], BF16, tag="tmpe")
        edb = pers.tile([128, S], F32, tag="edb")
        esc = pers.tile([128, NB, 1], F32, tag="esc")
        for i, n0 in enumerate(range(0, S, 512)):
            ptA = pse.tile([2, 512], F32, tag="ebp")
            ptB = pse.tile([2, 512], F32, tag="ebp2")
            for kk in range(4):
                for j, pt in ((0, ptA), (4, ptB)):
                    nc.tensor.matmul(pt[:], V16[:, j + kk, :], xT16[:, j + kk, n0:n0 + 512],
                                     start=(kk == 0), stop=(kk == 3))
            nc.vector.tensor_copy(out=tmpe[:, i, :], in_=ptA[:])
            nc.vector.tensor_tensor(eb2[:, n0:n0 + 512], tmpe[:, i, :], ptB[:], op=OP.add)
            # this half's e_dst broadcast and e_src transposes, so the Act
            # lrelu chain can start before the other half's matmuls finish
            ptd = psx.tile([128, 512], F32, tag="edbp")
            nc.tensor.matmul(ptd[:], sel[:], eb2[:, n0:n0 + 512], start=True, stop=True)
            nc.scalar.copy(out=edb[:, n0:n0 + 512], in_=ptd[:])
            for m in range(4 * i, 4 * i + 4):
                pt = psx.tile([128, 1], F32, tag="escp")
                nc.tensor.matmul(pt[:], eb2[0:1, m * 128:(m + 1) * 128], ones11[:],
                                 start=True, stop=True)
                nc.vector.tensor_copy(out=esc[:, m, :], in_=pt[:])

        # PE: h [node, feat] bf16, W_SCALE-scaled.  Two fp8 DoubleRow passes
        # (W-hi, then the natural-scale fp8 residual W-lo, which lands in the
        # denormal range) accumulate into ONE psum group — bf16-level weight
        # accuracy at fp8-DR speed, plain copy eviction.
        h16 = pers.tile([128, NB, H], BF16, tag="h16")
        for m in range(NB):
            for n0 in range(0, H, 512):
                pt = psum.tile([128, 512], F32, tag="mmp")
                for i, wt in enumerate((WTh16, WTl16)):
                    for d in range(ND):
                        nc.tensor.matmul(pt[:], xT16[:, 2 * d:2 * d + 2, m * 128:(m + 1) * 128],
                                         wt[:, 2 * d:2 * d + 2, n0:n0 + 512],
                                         start=(i == 0 and d == 0),
                                         stop=(i == 1 and d == ND - 1), perf_mode=DR)
                nc.vector.tensor_copy(out=h16[:, m, n0:n0 + 512], in_=pt[:])

        # Act: lrelu in column halves (starts as soon as the first edb half and
        # the matching esc blocks exist), then all exp; table loads: preloaded
        # Lrelu + one swap to Exp
        zl8 = pers.tile([128, NB, S], BF16, tag="zl8")
        for m in range(4):
            nc.scalar.activation(zl8[:, m, 0:512], edb[:, 0:512], AF.Lrelu,
                                 bias=esc[:, m, :], alpha=0.2)
        for m in range(4):
            nc.scalar.activation(zl8[:, m, 512:1024], edb[:, 512:1024], AF.Lrelu,
                                 bias=esc[:, m, :], alpha=0.2)
        for m in range(4, NB):
            nc.scalar.activation(zl8[:, m, 0:512], edb[:, 0:512], AF.Lrelu,
                                 bias=esc[:, m, :], alpha=0.2)
        for m in range(4, NB):
            nc.scalar.activation(zl8[:, m, 512:1024], edb[:, 512:1024], AF.Lrelu,
                                 bias=esc[:, m, :], alpha=0.2)
        ez8 = pers.tile([128, NB, S], BF16, tag="ez8")
        for m in range(NB):
            nc.scalar.activation(ez8[:, m, :], zl8[:, m, :], AF.Exp, scale=1.0 / V_SCALE)
        R = pers.tile([128, NB, S], BF16, tag="R")
        for m in range(2):
            nc.gpsimd.tensor_tensor(R[:, m, :], M0[:, m, :], ez8[:, m, :], op=OP.mult)
        for m in range(2, NB):
            nc.vector.tensor_tensor(R[:, m, :], M0[:, m, :], ez8[:, m, :], op=OP.mult)

        # PE: out^T [feat, t] = h^T R (raw; the host divides by attn); chunked DMA
        gsb = pers.tile([128, NB, S], BF16, tag="gsb")
        gTr = gT[:].rearrange("(m p) t -> p m t", p=128)
        for m in range(NB):
            for n0 in range(0, S, 512):
                pt = psum.tile([128, 512], F32, tag="mmp")
                for k in range(NB):
                    nc.tensor.matmul(pt[:], h16[:, k, m * 128:(m + 1) * 128],
                                     R[:, k, n0:n0 + 512],
                                     start=(k == 0), stop=(k == NB - 1))
                nc.vector.tensor_copy(out=gsb[:, m, n0:n0 + 512], in_=pt[:])
            nc.sync.dma_start(out=gTr[:, m, :], in_=gsb[:, m, :])

        # PE: attn^T [1, t] = 1^T R, after agg so it runs on a hot PE
        onesc = pers.tile([128, 1], BF16, tag="onesc")
        nc.vector.memset(onesc[:], 1.0)
        atT = pers.tile([1, S], F32, tag="atT")
        for n0 in range(0, S, 512):
            pt = pse.tile([1, 512], F32, tag="atp")
            for k in range(NB):
                nc.tensor.matmul(pt[:], onesc[:], R[:, k, n0:n0 + 512],
                                 start=(k == 0), stop=(k == NB - 1))
            nc.vector.tensor_copy(out=atT[:, n0:n0 + 512], in_=pt[:])
        nc.sync.dma_start(out=atO[:], in_=atT[:])
    nc.compile()
    return nc


def _build_D(nc):
    """Attention pool over nodes + 2-layer projection head, one batch per core."""
    x2T = nc.dram_tensor("x2T", [H, S], BF16, kind="ExternalInput")
    x2n = nc.dram_tensor("x2n", [S, H], BF16, kind="ExternalInput")
    wpc = nc.dram_tensor("wpc", [H, 1], BF16, kind="ExternalInput")
    w1T = nc.dram_tensor("w1T", [H, SEM], BF16, kind="ExternalInput")
    b1c = nc.dram_tensor("b1c", [SEM, 1], F32, kind="ExternalInput")
    w2T = nc.dram_tensor("w2T", [SEM, SEM], BF16, kind="ExternalInput")
    b2c = nc.dram_tensor("b2c", [SEM, 1], F32, kind="ExternalInput")
    res = nc.dram_tensor("res", [SEM, 1], F32, kind="ExternalOutput")

    with tile.TileContext(nc) as tc, ExitStack() as ctx:
        pers = ctx.enter_context(tc.tile_pool(name="pers", bufs=1))
        tmp = ctx.enter_context(tc.tile_pool(name="tmp", bufs=3))
        psum = ctx.enter_context(tc.tile_pool(name="psum", bufs=3, space="PSUM"))

        wp16 = pers.tile([128, NB, 1], BF16, tag="wp16")
        nc.sync.dma_start(out=wp16[:], in_=wpc[:].rearrange("(kb p) c -> p kb c", p=128))
        b1f = pers.tile([128, 4, 1], F32, tag="b1f")
        nc.sync.dma_start(out=b1f[:], in_=b1c[:].rearrange("(m p) c -> p m c", p=128))
        b2f = pers.tile([128, 4, 1], F32, tag="b2f")
        nc.sync.dma_start(out=b2f[:], in_=b2c[:].rearrange("(m p) c -> p m c", p=128))
        # x2T column-chunked so psc starts after the first quarter arrives
        x3T = pers.tile([128, NB, S], BF16, tag="x3T")
        x2Tr = x2T[:].rearrange("(kb p) s -> p kb s", p=128)
        for n0 in range(0, S, 256):
            nc.sync.dma_start(out=x3T[:, :, n0:n0 + 256], in_=x2Tr[:, :, n0:n0 + 256])
        x2t16 = pers.tile([128, NB, H], BF16, tag="x2t16")
        nc.sync.dma_start(out=x2t16[:], in_=x2n[:].rearrange("(tb p) f -> p tb f", p=128))
        w116 = pers.tile([128, NB, SEM], BF16, tag="w116")
        nc.sync.dma_start(out=w116[:], in_=w1T[:].rearrange("(kb p) c -> p kb c", p=128))
        w216 = pers.tile([128, 4, SEM], BF16, tag="w216")
        nc.sync.dma_start(out=w216[:], in_=w2T[:].rearrange("(kb p) c -> p kb c", p=128))

        # preload the Exp act table during the DMAs; warm the PE p-state
        warm = pers.tile([1, 1], F32, tag="warm")
        nc.vector.memset(warm[:], 0.0)
        nc.scalar.activation(warm[:], warm[:], AF.Exp)

        psc = pers.tile([1, S], F32, tag="psc")
        for n0 in range(0, S, 256):
            pt = psum.tile([1, 256], F32, tag="sp")
            for k in range(NB):
                nc.tensor.matmul(pt[:], wp16[:, k, :], x3T[:, k, n0:n0 + 256],
                                 start=(k == 0), stop=(k == NB - 1))
            nc.vector.tensor_copy(out=psc[:, n0:n0 + 256], in_=pt[:])

        mx = pers.tile([1, 1], F32, tag="mx")
        nc.vector.tensor_reduce(mx[:], psc[:], axis=AX.X, op=OP.max)
        nmx = pers.tile([1, 1], F32, tag="nmx")
        nc.vector.tensor_scalar(nmx[:], mx[:], -1.0, None, op0=OP.mult)
        ev = pers.tile([1, S], F32, tag="ev")
        nc.scalar.activation(ev[:], psc[:], AF.Exp, bias=nmx[:])
        sm = pers.tile([1, 1], F32, tag="sm")
        nc.vector.tensor_reduce(sm[:], ev[:], axis=AX.X, op=OP.add)
        rc = pers.tile([1, 1], F32, tag="rc")
        nc.vector.reciprocal(rc[:], sm[:])
        alT = pers.tile([1, S], BF16, tag="alT")
        nc.vector.tensor_scalar(alT[:], ev[:], rc[:], None, op0=OP.mult)

        # transpose alpha into partitions (8 tiny matmuls), then pooled = x2^T @ alpha
        # runs on PE instead of a serial DVE accumulation chain
        ones11 = pers.tile([1, 1], BF16, tag="ones11")
        nc.vector.memset(ones11[:], 1.0)
        alp = pers.tile([128, NB, 1], BF16, tag="alp")
        for tb in range(NB):
            pt = psum.tile([128, 1], F32, tag="sp1")
            nc.tensor.matmul(pt[:], alT[0:1, tb * 128:(tb + 1) * 128], ones11[:],
                             start=True, stop=True)
            nc.vector.tensor_copy(out=alp[:, tb, :], in_=pt[:])
        pld = pers.tile([128, NB, 1], BF16, tag="pld")
        for fb in range(NB):
            pt = psum.tile([128, 1], F32, tag="sp1")
            for tb in range(NB):
                nc.tensor.matmul(pt[:], x2t16[:, tb, fb * 128:(fb + 1) * 128], alp[:, tb, :],
                                 start=(tb == 0), stop=(tb == NB - 1))
            nc.vector.tensor_copy(out=pld[:, fb, :], in_=pt[:])

        hid = pers.tile([128, 4, 1], BF16, tag="hid")
        for m in range(4):
            pt = psum.tile([128, 1], F32, tag="sp1")
            for k in range(NB):
                nc.tensor.matmul(pt[:], w116[:, k, m * 128:(m + 1) * 128], pld[:, k, :],
                                 start=(k == 0), stop=(k == NB - 1))
            nc.scalar.activation(hid[:, m, :], pt[:], AF.Relu, bias=b1f[:, m, :])

        rsb = pers.tile([128, 4, 1], F32, tag="rsb")
        for m in range(4):
            pt = psum.tile([128, 1], F32, tag="sp1")
            for k in range(4):
                nc.tensor.matmul(pt[:], w216[:, k, m * 128:(m + 1) * 128], hid[:, k, :],
                                 start=(k == 0), stop=(k == 3))
            nc.vector.tensor_tensor(rsb[:, m, :], pt[:], b2f[:, m, :], op=OP.add)
        nc.sync.dma_start(out=res[:].rearrange("(m p) c -> p m c", p=128), in_=rsb[:])
    nc.compile()
    return nc


_PROGS = {}


def _get_progs():
    if not _PROGS:
        def mk():
            return bacc.Bacc("TRN2", target_bir_lowering=False, debug=False,
                             enable_asserts=True, num_devices=8)
        _PROGS["A0"] = _build_P0(mk())
        _PROGS["A"] = _build_P1(mk())
        _PROGS["B"] = _build_L(mk())
        _PROGS["C"] = _build_L(mk())
        _PROGS["D"] = _build_D(mk())
    return _PROGS


def kernel(hidden_states, phi_w, psi_w, gat_lin_w, gat_att, wp, w1, b1, w2, b2,
           _profile=None):
    f32 = np.float32
    bf16 = ml_dtypes.bfloat16
    hidden_states = np.asarray(hidden_states, f32)
    progs = _get_progs()
    C = lambda a: np.ascontiguousarray(a)
    times = {}

    def run(tag, in_maps, core_ids):
        r = run_bass_kernel_spmd(progs[tag], in_maps, core_ids=core_ids)
        if _profile is not None:
            times[tag] = r.exec_time_ns
        return r.results

    f8 = ml_dtypes.float8_e4m3
    glw = np.asarray(gat_lin_w, f32)
    ga = np.asarray(gat_att, f32)
    xTb = [C(hidden_states[b].T.astype(bf16)) for b in range(B)]
    xTb8 = [C(hidden_states[b].T.astype(f8)) for b in range(B)]

    # ---- launch P0: M = phi_w.T @ psi_w chunks, V = W^T [a_src|a_dst] ----
    # (reference einsum 'bsd,ed->bse' is x @ phi_w.T, so scores = x M x.T with
    # M = phi_w.T @ psi_w; the contraction runs over the e rows of both.)
    pT = np.asarray(phi_w, f32).astype(bf16)
    sT = C(np.asarray(psi_w, f32).astype(bf16))
    in_0 = []
    for c in range(8):
        l, hd = c // 4, c % 4
        in_0.append({
            "pTc": C(pT[:, c * 128:(c + 1) * 128]),
            "sT": sT,
            "Wn": C((glw[l, hd * H:(hd + 1) * H, :] * W_SCALE).astype(f8)),
            "a2": C((np.stack([ga[l, hd, :H], ga[l, hd, H:]], axis=1) * A_SCALE).astype(f8)),
        })
    r0 = run("A0", in_0, list(range(8)))
    Mfull = C(np.concatenate([r0[c]["Mc"] for c in range(8)], axis=0))
    V2 = [[C((r0[l * 4 + hd]["VT"].T * V_SCALE).astype(f8)) for hd in range(4)] for l in range(2)]

    # ---- launch P1: edge build ----
    in_a = []
    for c in range(8):
        b, rcn = c // 4, c % 4
        in_a.append({
            "xT": xTb[b], "xTc": C(xTb[b][:, rcn * CH:(rcn + 1) * CH]),
            "Mm": Mfull,
            "srcx": C(np.arange(rcn * CH, (rcn + 1) * CH, dtype=np.float32)[:, None]),
        })
    ra = run("A", in_a, list(range(8)))
    topi = np.stack([np.concatenate([ra[b * 4 + r]["topi"] for r in range(4)], 0) for b in range(B)])
    ew = np.stack([np.concatenate([ra[b * 4 + r]["ew"] for r in range(4)], 0) for b in range(B)])
    tpi16 = [C(topi[b].astype(np.int16)) for b in range(B)]
    ew16 = [C(ew[b].astype(bf16)) for b in range(B)]

    # ---- launches P2, P3: the two GAT layers (host pre-sums partials) ----
    sel2 = C(np.stack([np.zeros(128, f32), np.ones(128, f32)]).astype(bf16))
    xin8 = xTb8
    for li, tag in enumerate(("B", "C")):
        in_l = []
        for c in range(8):
            b, hd = c // 4, c % 4
            w32 = glw[li, hd * H:(hd + 1) * H, :].T * W_SCALE
            wh = w32.astype(f8)
            wl = (w32 - wh.astype(f32)).astype(f8)
            in_l.append({
                "xT": xin8[b],
                "WTh": C(wh), "WTl": C(wl),
                "V2": V2[li][hd],
                "tpi": tpi16[b], "ewd": ew16[b],
                "sl2": sel2,
            })
        rl = run(tag, in_l, list(range(8)))
        xin8, xacc = [], []
        for b in range(B):
            acc = sum(rl[b * 4 + i]["gT"].astype(f32) / (rl[b * 4 + i]["atO"][0] + 1e-8)
                      for i in range(4)) / (HEADS * W_SCALE)
            acc = np.maximum(acc, 0.0)
            xacc.append(acc)
            xin8.append(C(acc.astype(f8)))

    # ---- launch P4: pooling + projection head ----
    in_d = []
    for b in range(B):
        in_d.append({
            "x2T": C(xacc[b].astype(bf16)),
            "x2n": C(xacc[b].T.astype(bf16)),
            "wpc": C(np.asarray(wp, f32).reshape(H, 1).astype(bf16)),
            "w1T": C(np.asarray(w1, f32).T.astype(bf16)),
            "b1c": C(np.asarray(b1, f32)[:, None]),
            "w2T": C(np.asarray(w2, f32).T.astype(bf16)),
            "b2c": C(np.asarray(b2, f32)[:, None]),
        })
    rd = run("D", in_d, [0, 1])
    out = np.stack([rd[b]["res"][:, 0].astype(f32) for b in range(B)])
    if _profile is not None:
        _profile.update(times)
    return out


# revision 59
# speedup vs baseline: 1.0019x; 1.0019x over previous
"""Trainium2 Bass kernel for nn_GraphSemanticExtractor (GNN message passing).

Sharding (8 NeuronCores), 5 launches with host-side layout glue between them:
  P0: core c => 128-row chunk of M = phi_w @ psi_w.T, plus V = W^T [a_src|a_dst]
      for (layer l=c//4, head hd=c%4).
  P1 (edge build): core c => (batch b=c//4, 256-row chunk rc=c%4);
      scores = (x_c @ M) @ x.T, top-8, softmax over the 8, self-edge mask.
  P2/P3 (GAT layers 1/2): core c => (batch b=c//4, head hd=c%4); between the
      two, the host computes x1 = relu(sum of per-head partials).
  P4: pool + projection head, core b in {0,1}.

The sparse top-k aggregation out[dst] += wgt*h[src] is a dense matmul
out.T = h.T @ R with R[s,t] = ew_k(s)*exp(lrelu(e_src[s]+e_dst[t])) at
t=topi[s,k]; R is built by scattering ew into M0 (gpsimd local_scatter) and a
dense lrelu/exp of the rank-1 e-grid, all overlapped with the h matmul on PE.
"""

import sys

sys.path.insert(0, "/opt/trn_rl_repo")
sys.path.insert(0, "/opt/trn_rl_repo/concourse")

from contextlib import ExitStack

import ml_dtypes
import numpy as np

import concourse.bass as bass
import concourse.tile as tile
from concourse import bacc, mybir
from concourse.bass_utils import run_bass_kernel_spmd

F32 = mybir.dt.float32
BF16 = mybir.dt.bfloat16
F8 = mybir.dt.float8e4
U32 = mybir.dt.uint32
I16 = mybir.dt.int16
AF = mybir.ActivationFunctionType
OP = mybir.AluOpType
AX = mybir.AxisListType
DR = mybir.MatmulPerfMode.DoubleRow

B, S, H = 2, 1024, 1024
HEADS, K = 4, 8
SEM = 512
NB = H // 128  # 8 partition blocks
ND = NB // 2   # 4 double-row blocks for fp8 DoubleRow matmuls
CH = S // 4    # 256 rows per edge-build core

# fp8 e4m3 has min-normal 2^-6; the tiny GAT weights (~0.02 scale) are scaled
# up on the host and the factors folded back into on-device scalars.
W_SCALE = 32.0   # W.T fed to the h matmul
V_SCALE = 64.0   # V = W^T [a_src|a_dst] fed to the e matmuls
A_SCALE = 64.0   # gat_att halves fed to P0's V matmul


def _build_P0(nc):
    """Per core: 128 rows of M = phi_w.T @ psi_w, and V = W^T [a_src|a_dst]
    for one (layer, head).  The V matmul runs in fp8 DoubleRow (host scales
    its inputs by A_SCALE*W_SCALE; the eviction scales back)."""
    pTc = nc.dram_tensor("pTc", [H, 128], BF16, kind="ExternalInput")
    sT = nc.dram_tensor("sT", [H, H], BF16, kind="ExternalInput")
    Wn = nc.dram_tensor("Wn", [H, H], F8, kind="ExternalInput")
    a2 = nc.dram_tensor("a2", [H, 2], F8, kind="ExternalInput")
    Mc = nc.dram_tensor("Mc", [128, H], BF16, kind="ExternalOutput")
    VT = nc.dram_tensor("VT", [2, H], F32, kind="ExternalOutput")

    with tile.TileContext(nc) as tc, ExitStack() as ctx:
        pers = ctx.enter_context(tc.tile_pool(name="pers", bufs=1))
        psum = ctx.enter_context(tc.tile_pool(name="psum", bufs=4, space="PSUM"))

        # all input DMAs on the sync queue, smallest first (single DMA engine
        # processes FIFO; a big transfer queued first would stall the rest)
        a2t = pers.tile([128, NB, 2], F8, tag="a2t")
        nc.sync.dma_start(out=a2t[:], in_=a2[:].rearrange("(kb p) c -> p kb c", p=128))
        pT16 = pers.tile([128, NB, 128], BF16, tag="pT16")
        nc.sync.dma_start(out=pT16[:], in_=pTc[:].rearrange("(kb p) c -> p kb c", p=128))
        Wn16 = pers.tile([128, NB, H], F8, tag="Wn16")
        nc.sync.dma_start(out=Wn16[:], in_=Wn[:].rearrange("(kb p) s -> p kb s", p=128))
        sT16 = pers.tile([128, NB, H], BF16, tag="sT16")
        nc.sync.dma_start(out=sT16[:], in_=sT[:].rearrange("(kb p) s -> p kb s", p=128))

        Vt = pers.tile([2, H], F32, tag="Vt")
        for n0 in range(0, H, 512):
            pt = psum.tile([2, 512], F32, tag="vm")
            for k in range(NB):
                nc.tensor.matmul(pt[:], a2t[:, k, :], Wn16[:, k, n0:n0 + 512],
                                 start=(k == 0), stop=(k == NB - 1))
            nc.vector.tensor_scalar(Vt[:, n0:n0 + 512], pt[:],
                                    1.0 / (A_SCALE * W_SCALE), None, op0=OP.mult)
        nc.sync.dma_start(out=VT[:], in_=Vt[:])

        Mc16 = pers.tile([128, H], BF16, tag="Mc16")
        for n0 in range(0, H, 512):
            pt = psum.tile([128, 512], F32, tag="mm")
            for k in range(NB):
                nc.tensor.matmul(pt[:], pT16[:, k, :], sT16[:, k, n0:n0 + 512],
                                 start=(k == 0), stop=(k == NB - 1))
            nc.vector.tensor_copy(out=Mc16[:, n0:n0 + 512], in_=pt[:])
        nc.sync.dma_start(out=Mc[:], in_=Mc16[:])
    nc.compile()
    return nc


def _build_P1(nc):
    """Edge build: scores = (x_c @ M) @ x.T, top-8 + softmax + self-mask."""
    xT = nc.dram_tensor("xT", [H, S], BF16, kind="ExternalInput")
    xTc = nc.dram_tensor("xTc", [H, CH], BF16, kind="ExternalInput")
    Mm = nc.dram_tensor("Mm", [H, H], BF16, kind="ExternalInput")
    srcx = nc.dram_tensor("srcx", [CH, 1], F32, kind="ExternalInput")
    topi = nc.dram_tensor("topi", [CH, K], U32, kind="ExternalOutput")
    ew = nc.dram_tensor("ew", [CH, K], F32, kind="ExternalOutput")

    with tile.TileContext(nc) as tc, ExitStack() as ctx:
        pers = ctx.enter_context(tc.tile_pool(name="pers", bufs=1))
        psum = ctx.enter_context(tc.tile_pool(name="psum", bufs=4, space="PSUM"))
        psumb = ctx.enter_context(tc.tile_pool(name="psumb", bufs=4, space="PSUM"))

        sx = pers.tile([128, 2, 1], F32, tag="sx")
        nc.sync.dma_start(out=sx[:], in_=srcx[:].rearrange("(m p) c -> p m c", p=128))
        xTc16 = pers.tile([128, NB, CH], BF16, tag="xTc16")
        nc.sync.dma_start(out=xTc16[:], in_=xTc[:].rearrange("(kb p) s -> p kb s", p=128))
        M16 = pers.tile([128, NB, H], BF16, tag="M16")
        nc.sync.dma_start(out=M16[:], in_=Mm[:].rearrange("(kb p) s -> p kb s", p=128))
        xT16 = pers.tile([128, NB, S], BF16, tag="xT16")
        nc.sync.dma_start(out=xT16[:], in_=xT[:].rearrange("(kb p) s -> p kb s", p=128))

        # preload the Exp act table while DMAs run so the top-k chain's exp
        # doesn't pay the 1.3us table load; warm the PE p-state too
        warm = pers.tile([1, 1], F32, tag="warm")
        nc.vector.memset(warm[:], 0.0)
        nc.scalar.activation(warm[:], warm[:], AF.Exp)

        # PT[j, s-chunk] = (x_c @ M).T  (j = feature of M's column space)
        PT16 = pers.tile([128, NB, CH], BF16, tag="PT16")
        for m in range(NB):
            pt = psumb.tile([128, CH], F32, tag="ptm")
            for k in range(NB):
                nc.tensor.matmul(pt[:], M16[:, k, m * 128:(m + 1) * 128], xTc16[:, k, :],
                                 start=(k == 0), stop=(k == NB - 1))
            nc.vector.tensor_copy(out=PT16[:, m, :], in_=pt[:])

        # scores [s-chunk, t] f32; the whole per-sb top-8/softmax/mask chain runs
        # while the other sb-block's matmuls occupy PE
        sc = pers.tile([128, 2, S], F32, tag="scores")
        mv = pers.tile([128, 2, K], F32, tag="mv")
        ti = pers.tile([128, 2, K], U32, tag="ti")
        ex = pers.tile([128, 2, K], F32, tag="ex")
        sm = pers.tile([128, 2, 1], F32, tag="sm")
        rc = pers.tile([128, 2, 1], F32, tag="rc")
        tif = pers.tile([128, 2, K], F32, tag="tif")
        w8 = pers.tile([128, 2, K], F32, tag="w8")
        msk = pers.tile([128, 2, K], F32, tag="msk")
        ewt = pers.tile([128, 2, K], F32, tag="ewt")
        topir = topi[:].rearrange("(m p) k -> p m k", p=128)
        ewr = ew[:].rearrange("(m p) k -> p m k", p=128)
        for sb in range(2):
            for n0 in range(0, S, 512):
                pt = psum.tile([128, 512], F32, tag="scm")
                for k in range(NB):
                    nc.tensor.matmul(pt[:], PT16[:, k, sb * 128:(sb + 1) * 128],
                                     xT16[:, k, n0:n0 + 512],
                                     start=(k == 0), stop=(k == NB - 1))
                nc.vector.tensor_copy(out=sc[:, sb, n0:n0 + 512], in_=pt[:])
            nc.vector.max(mv[:, sb, :], sc[:, sb, :])
            nc.vector.max_index(ti[:, sb, :], mv[:, sb, :], sc[:, sb, :])
            nc.sync.dma_start(out=topir[:, sb, :], in_=ti[:, sb, :])
            nc.scalar.activation(ex[:, sb, :], mv[:, sb, :], AF.Exp)
            nc.vector.tensor_reduce(sm[:, sb, :], ex[:, sb, :], axis=AX.X, op=OP.add)
            nc.vector.tensor_scalar(sm[:, sb, :], sm[:, sb, :], 1e-8, None, op0=OP.add)
            nc.vector.reciprocal(rc[:, sb, :], sm[:, sb, :])
            nc.vector.tensor_copy(out=tif[:, sb, :], in_=ti[:, sb, :])
            nc.vector.tensor_scalar(w8[:, sb, :], ex[:, sb, :], rc[:, sb, :], 1e-8, op0=OP.mult, op1=OP.max)
            nc.vector.tensor_scalar(msk[:, sb, :], tif[:, sb, :], sx[:, sb, :], None, op0=OP.is_equal)
            nc.vector.tensor_scalar(msk[:, sb, :], msk[:, sb, :], -1.0, 1.0, op0=OP.mult, op1=OP.add)
            nc.vector.tensor_tensor(ewt[:, sb, :], w8[:, sb, :], msk[:, sb, :], op=OP.mult)
            nc.sync.dma_start(out=ewr[:, sb, :], in_=ewt[:, sb, :])
    nc.compile()
    return nc


def _build_L(nc):
    """One GAT layer for one (batch, head).  gT[feat, node] = (agg/attn)/HEADS.
    The h matmul runs fp8 DoubleRow with W split into an fp8 hi/lo residual
    pair (hi + lo/16 restores ~bf16 weight accuracy; fp8 W alone costs 2e-2
    output error).  x and V are plain fp8 (~2e-3 each).  R and the
    aggregation stay bf16: fp8 R alone costs 4e-2."""
    xT = nc.dram_tensor("xT", [H, S], F8, kind="ExternalInput")
    WTh = nc.dram_tensor("WTh", [H, H], F8, kind="ExternalInput")
    WTl = nc.dram_tensor("WTl", [H, H], F8, kind="ExternalInput")
    V2 = nc.dram_tensor("V2", [H, 2], F8, kind="ExternalInput")
    tpi = nc.dram_tensor("tpi", [S, K], I16, kind="ExternalInput")
    ewd = nc.dram_tensor("ewd", [S, K], BF16, kind="ExternalInput")
    sl2 = nc.dram_tensor("sl2", [2, 128], BF16, kind="ExternalInput")
    gT = nc.dram_tensor("gT", [H, S], BF16, kind="ExternalOutput")
    atO = nc.dram_tensor("atO", [1, S], F32, kind="ExternalOutput")

    with tile.TileContext(nc) as tc, ExitStack() as ctx:
        pers = ctx.enter_context(tc.tile_pool(name="pers", bufs=1))
        psum = ctx.enter_context(tc.tile_pool(name="psum", bufs=2, space="PSUM"))
        pse = ctx.enter_context(tc.tile_pool(name="pse", bufs=1, space="PSUM"))
        psx = ctx.enter_context(tc.tile_pool(name="psx", bufs=1, space="PSUM"))

        # all inputs on the sync queue, smallest first (FIFO DMA engine)
        sel = pers.tile([2, 128], BF16, tag="sel")
        nc.sync.dma_start(out=sel[:], in_=sl2[:])
        V16 = pers.tile([128, NB, 2], F8, tag="V16")
        nc.sync.dma_start(out=V16[:], in_=V2[:].rearrange("(kb p) c -> p kb c", p=128))
        tpw = pers.tile([128, NB, K], I16, tag="tpw")
        nc.sync.dma_start(out=tpw[:], in_=tpi[:].rearrange("(m p) k -> p m k", p=128))
        ews16 = pers.tile([128, NB, K], BF16, tag="ews16")
        nc.sync.dma_start(out=ews16[:], in_=ewd[:].rearrange("(m p) k -> p m k", p=128))
        xT16 = pers.tile([128, NB, S], F8, tag="xT16")  # noqa: dma order: tiny first
        nc.sync.dma_start(out=xT16[:], in_=xT[:].rearrange("(kb p) s -> p kb s", p=128))
        WTh16 = pers.tile([128, NB, H], F8, tag="WTh16")
        nc.sync.dma_start(out=WTh16[:], in_=WTh[:].rearrange("(kb p) s -> p kb s", p=128))
        WTl16 = pers.tile([128, NB, H], F8, tag="WTl16")
        nc.sync.dma_start(out=WTl16[:], in_=WTl[:].rearrange("(kb p) s -> p kb s", p=128))

        ones11 = pers.tile([1, 1], BF16, tag="ones11")
        nc.vector.memset(ones11[:], 1.0)
        # preload the Lrelu act table while DMAs run
        warm = pers.tile([1, 1], F32, tag="warm")
        nc.vector.memset(warm[:], 0.0)
        nc.scalar.activation(warm[:], warm[:], AF.Lrelu, alpha=0.2)

        # gpsimd: M0 blocks (scatter of ew into dense [s, t])
        M0 = pers.tile([128, NB, S], BF16, tag="M0")
        for m in range(NB):
            nc.gpsimd.local_scatter(M0[:, m, :], ews16[:, m, :], tpw[:, m, :],
                                    channels=128, num_elems=S, num_idxs=K)

        # PE: e_bothT [2, node] = V^T x as two independent k-split psum groups
        # (independent groups queue up and run at ramped PE speed); eb2 keeps
        # the V_SCALE factor — the exp pass divides it out via its scale param
        # (lrelu is positively homogeneous so the order is exact)
        eb2 = pers.tile([2, S], BF16, tag="eb2")
        tmpe = pers.tile([2, 2, 512], BF16, tag="tmpe")
        edb = pers.tile([128, S], F32, tag="edb")
        esc = pers.tile([128, NB, 1], F32, tag="esc")
        for i, n0 in enumerate(range(0, S, 512)):
            ptA = pse.tile([2, 512], F32, tag="ebp")
            ptB = pse.tile([2, 512], F32, tag="ebp2")
            for kk in range(4):
                for j, pt in ((0, ptA), (4, ptB)):
                    nc.tensor.matmul(pt[:], V16[:, j + kk, :], xT16[:, j + kk, n0:n0 + 512],
                                     start=(kk == 0), stop=(kk == 3))
            nc.vector.tensor_copy(out=tmpe[:, i, :], in_=ptA[:])
            nc.vector.tensor_tensor(eb2[:, n0:n0 + 512], tmpe[:, i, :], ptB[:], op=OP.add)
            # this half's e_dst broadcast and e_src transposes, so the Act
            # lrelu chain can start before the other half's matmuls finish
            ptd = psx.tile([128, 512], F32, tag="edbp")
            nc.tensor.matmul(ptd[:], sel[:], eb2[:, n0:n0 + 512], start=True, stop=True)
            nc.scalar.copy(out=edb[:, n0:n0 + 512], in_=ptd[:])
            for m in range(4 * i, 4 * i + 4):
                pt = psx.tile([128, 1], F32, tag="escp")
                nc.tensor.matmul(pt[:], eb2[0:1, m * 128:(m + 1) * 128], ones11[:],
                                 start=True, stop=True)
                nc.vector.tensor_copy(out=esc[:, m, :], in_=pt[:])

        # PE: h [node, feat] bf16, W_SCALE-scaled.  Two fp8 DoubleRow passes
        # (W-hi, then the natural-scale fp8 residual W-lo, which lands in the
        # denormal range) accumulate into ONE psum group — bf16-level weight
        # accuracy at fp8-DR speed, plain copy eviction.
        h16 = pers.tile([128, NB, H], BF16, tag="h16")
        for m in range(NB):
            for n0 in range(0, H, 512):
                pt = psum.tile([128, 512], F32, tag="mmp")
                for i, wt in enumerate((WTh16, WTl16)):
                    for d in range(ND):
                        nc.tensor.matmul(pt[:], xT16[:, 2 * d:2 * d + 2, m * 128:(m + 1) * 128],
                                         wt[:, 2 * d:2 * d + 2, n0:n0 + 512],
                                         start=(i == 0 and d == 0),
                                         stop=(i == 1 and d == ND - 1), perf_mode=DR)
                nc.vector.tensor_copy(out=h16[:, m, n0:n0 + 512], in_=pt[:])

        # Act: lrelu in column halves (starts as soon as the first edb half and
        # the matching esc blocks exist), then all exp; table loads: preloaded
        # Lrelu + one swap to Exp
        zl8 = pers.tile([128, NB, S], BF16, tag="zl8")
        for m in range(4):
            nc.scalar.activation(zl8[:, m, 0:512], edb[:, 0:512], AF.Lrelu,
                                 bias=esc[:, m, :], alpha=0.2)
        for m in range(4):
            nc.scalar.activation(zl8[:, m, 512:1024], edb[:, 512:1024], AF.Lrelu,
                                 bias=esc[:, m, :], alpha=0.2)
        for m in range(4, NB):
            nc.scalar.activation(zl8[:, m, 0:512], edb[:, 0:512], AF.Lrelu,
                                 bias=esc[:, m, :], alpha=0.2)
        for m in range(4, NB):
            nc.scalar.activation(zl8[:, m, 512:1024], edb[:, 512:1024], AF.Lrelu,
                                 bias=esc[:, m, :], alpha=0.2)
        ez8 = pers.tile([128, NB, S], BF16, tag="ez8")
        for m in range(NB):
            nc.scalar.activation(ez8[:, m, :], zl8[:, m, :], AF.Exp, scale=1.0 / V_SCALE)
        R = pers.tile([128, NB, S], BF16, tag="R")
        for m in range(2):
            nc.gpsimd.tensor_tensor(R[:, m, :], M0[:, m, :], ez8[:, m, :], op=OP.mult)
        for m in range(2, NB):
            nc.vector.tensor_tensor(R[:, m, :], M0[:, m, :], ez8[:, m, :], op=OP.mult)

        # PE: out^T [feat, t] = h^T R (raw; the host divides by attn); chunked DMA
        gsb = pers.tile([128, NB, S], BF16, tag="gsb")
        gTr = gT[:].rearrange("(m p) t -> p m t", p=128)
        for m in range(NB):
            for n0 in range(0, S, 512):
                pt = psum.tile([128, 512], F32, tag="mmp")
                for k in range(NB):
                    nc.tensor.matmul(pt[:], h16[:, k, m * 128:(m + 1) * 128],
                                     R[:, k, n0:n0 + 512],
                                     start=(k == 0), stop=(k == NB - 1))
                nc.vector.tensor_copy(out=gsb[:, m, n0:n0 + 512], in_=pt[:])
            nc.sync.dma_start(out=gTr[:, m, :], in_=gsb[:, m, :])

        # PE: attn^T [1, t] = 1^T R, after agg so it runs on a hot PE
        onesc = pers.tile([128, 1], BF16, tag="onesc")
        nc.vector.memset(onesc[:], 1.0)
        atT = pers.tile([1, S], F32, tag="atT")
        for n0 in range(0, S, 512):
            pt = pse.tile([1, 512], F32, tag="atp")
            for k in range(NB):
                nc.tensor.matmul(pt[:], onesc[:], R[:, k, n0:n0 + 512],
                                 start=(k == 0), stop=(k == NB - 1))
            nc.vector.tensor_copy(out=atT[:, n0:n0 + 512], in_=pt[:])
        nc.sync.dma_start(out=atO[:], in_=atT[:])
    nc.compile()
    return nc


def _build_D(nc):
    """Attention pool over nodes + 2-layer projection head, one batch per core."""
    x2T = nc.dram_tensor("x2T", [H, S], BF16, kind="ExternalInput")
    x2n = nc.dram_tensor("x2n", [S, H], BF16, kind="ExternalInput")
    wpc = nc.dram_tensor("wpc", [H, 1], BF16, kind="ExternalInput")
    w1T = nc.dram_tensor("w1T", [H, SEM], BF16, kind="ExternalInput")
    b1c = nc.dram_tensor("b1c", [SEM, 1], F32, kind="ExternalInput")
    w2T = nc.dram_tensor("w2T", [SEM, SEM], BF16, kind="ExternalInput")
    b2c = nc.dram_tensor("b2c", [SEM, 1], F32, kind="ExternalInput")
    res = nc.dram_tensor("res", [SEM, 1], F32, kind="ExternalOutput")

    with tile.TileContext(nc) as tc, ExitStack() as ctx:
        pers = ctx.enter_context(tc.tile_pool(name="pers", bufs=1))
        tmp = ctx.enter_context(tc.tile_pool(name="tmp", bufs=3))
        psum = ctx.enter_context(tc.tile_pool(name="psum", bufs=3, space="PSUM"))

        wp16 = pers.tile([128, NB, 1], BF16, tag="wp16")
        nc.sync.dma_start(out=wp16[:], in_=wpc[:].rearrange("(kb p) c -> p kb c", p=128))
        b1f = pers.tile([128, 4, 1], F32, tag="b1f")
        nc.sync.dma_start(out=b1f[:], in_=b1c[:].rearrange("(m p) c -> p m c", p=128))
        b2f = pers.tile([128, 4, 1], F32, tag="b2f")
        nc.sync.dma_start(out=b2f[:], in_=b2c[:].rearrange("(m p) c -> p m c", p=128))
        # x2T column-chunked so psc starts after the first quarter arrives
        x3T = pers.tile([128, NB, S], BF16, tag="x3T")
        x2Tr = x2T[:].rearrange("(kb p) s -> p kb s", p=128)
        for n0 in range(0, S, 256):
            nc.sync.dma_start(out=x3T[:, :, n0:n0 + 256], in_=x2Tr[:, :, n0:n0 + 256])
        x2t16 = pers.tile([128, NB, H], BF16, tag="x2t16")
        nc.sync.dma_start(out=x2t16[:], in_=x2n[:].rearrange("(tb p) f -> p tb f", p=128))
        w116 = pers.tile([128, NB, SEM], BF16, tag="w116")
        nc.sync.dma_start(out=w116[:], in_=w1T[:].rearrange("(kb p) c -> p kb c", p=128))
        w216 = pers.tile([128, 4, SEM], BF16, tag="w216")
        nc.sync.dma_start(out=w216[:], in_=w2T[:].rearrange("(kb p) c -> p kb c", p=128))

        # preload the Exp act table during the DMAs; warm the PE p-state
        warm = pers.tile([1, 1], F32, tag="warm")
        nc.vector.memset(warm[:], 0.0)
        nc.scalar.activation(warm[:], warm[:], AF.Exp)

        psc = pers.tile([1, S], F32, tag="psc")
        for n0 in range(0, S, 256):
            pt = psum.tile([1, 256], F32, tag="sp")
            for k in range(NB):
                nc.tensor.matmul(pt[:], wp16[:, k, :], x3T[:, k, n0:n0 + 256],
                                 start=(k == 0), stop=(k == NB - 1))
            nc.vector.tensor_copy(out=psc[:, n0:n0 + 256], in_=pt[:])

        mx = pers.tile([1, 1], F32, tag="mx")
        nc.vector.tensor_reduce(mx[:], psc[:], axis=AX.X, op=OP.max)
        nmx = pers.tile([1, 1], F32, tag="nmx")
        nc.vector.tensor_scalar(nmx[:], mx[:], -1.0, None, op0=OP.mult)
        ev = pers.tile([1, S], F32, tag="ev")
        nc.scalar.activation(ev[:], psc[:], AF.Exp, bias=nmx[:])
        sm = pers.tile([1, 1], F32, tag="sm")
        nc.vector.tensor_reduce(sm[:], ev[:], axis=AX.X, op=OP.add)
        rc = pers.tile([1, 1], F32, tag="rc")
        nc.vector.reciprocal(rc[:], sm[:])
        alT = pers.tile([1, S], BF16, tag="alT")
        nc.vector.tensor_scalar(alT[:], ev[:], rc[:], None, op0=OP.mult)

        # transpose alpha into partitions (8 tiny matmuls), then pooled = x2^T @ alpha
        # runs on PE instead of a serial DVE accumulation chain
        ones11 = pers.tile([1, 1], BF16, tag="ones11")
        nc.vector.memset(ones11[:], 1.0)
        alp = pers.tile([128, NB, 1], BF16, tag="alp")
        for tb in range(NB):
            pt = psum.tile([128, 1], F32, tag="sp1")
            nc.tensor.matmul(pt[:], alT[0:1, tb * 128:(tb + 1) * 128], ones11[:],
                             start=True, stop=True)
            nc.vector.tensor_copy(out=alp[:, tb, :], in_=pt[:])
        pld = pers.tile([128, NB, 1], BF16, tag="pld")
        for fb in range(NB):
            pt = psum.tile([128, 1], F32, tag="sp1")
            for tb in range(NB):
                nc.tensor.matmul(pt[:], x2t16[:, tb, fb * 128:(fb + 1) * 128], alp[:, tb, :],
                                 start=(tb == 0), stop=(tb == NB - 1))
            nc.vector.tensor_copy(out=pld[:, fb, :], in_=pt[:])

        hid = pers.tile([128, 4, 1], BF16, tag="hid")
        for m in range(4):
            pt = psum.tile([128, 1], F32, tag="sp1")
            for k in range(NB):
                nc.tensor.matmul(pt[:], w116[:, k, m * 128:(m + 1) * 128], pld[:, k, :],
                                 start=(k == 0), stop=(k == NB - 1))
            nc.scalar.activation(hid[:, m, :], pt[:], AF.Relu, bias=b1f[:, m, :])

        rsb = pers.tile([128, 4, 1], F32, tag="rsb")
        for m in range(4):
            pt = psum.tile([128, 1], F32, tag="sp1")
            for k in range(4):
                nc.tensor.matmul(pt[:], w216[:, k, m * 128:(m + 1) * 128], hid[:, k, :],
                                 start=(k == 0), stop=(k == 3))
            nc.vector.tensor_tensor(rsb[:, m, :], pt[:], b2f[:, m, :], op=OP.add)
        nc.sync.dma_start(out=res[:].rearrange("(m p) c -> p m c", p=128), in_=rsb[:])
    nc.compile()
    return nc


_PROGS = {}


def _get_progs():
    if not _PROGS:
        def mk():
            return bacc.Bacc("TRN2", target_bir_lowering=False, debug=False,
                             enable_asserts=True, num_devices=8)
        _PROGS["A0"] = _build_P0(mk())
        _PROGS["A"] = _build_P1(mk())
        _PROGS["B"] = _build_L(mk())
        _PROGS["C"] = _build_L(mk())
        _PROGS["D"] = _build_D(mk())
    return _PROGS


def kernel(hidden_states, phi_w, psi_w, gat_lin_w, gat_att, wp, w1, b1, w2, b2,
           _profile=None):
    f32 = np.float32
    bf16 = ml_dtypes.bfloat16
    hidden_states = np.asarray(hidden_states, f32)
    progs = _get_progs()
    C = lambda a: np.ascontiguousarray(a)
    times = {}

    def run(tag, in_maps, core_ids):
        r = run_bass_kernel_spmd(progs[tag], in_maps, core_ids=core_ids)
        if _profile is not None:
            times[tag] = r.exec_time_ns
        return r.results

    f8 = ml_dtypes.float8_e4m3
    glw = np.asarray(gat_lin_w, f32)
    ga = np.asarray(gat_att, f32)
    xTb = [C(hidden_states[b].T.astype(bf16)) for b in range(B)]
    xTb8 = [C(hidden_states[b].T.astype(f8)) for b in range(B)]

    # ---- launch P0: M = phi_w.T @ psi_w chunks, V = W^T [a_src|a_dst] ----
    # (reference einsum 'bsd,ed->bse' is x @ phi_w.T, so scores = x M x.T with
    # M = phi_w.T @ psi_w; the contraction runs over the e rows of both.)
    pT = np.asarray(phi_w, f32).astype(bf16)
    sT = C(np.asarray(psi_w, f32).astype(bf16))
    in_0 = []
    for c in range(8):
        l, hd = c // 4, c % 4
        in_0.append({
            "pTc": C(pT[:, c * 128:(c + 1) * 128]),
            "sT": sT,
            "Wn": C((glw[l, hd * H:(hd + 1) * H, :] * W_SCALE).astype(f8)),
            "a2": C((np.stack([ga[l, hd, :H], ga[l, hd, H:]], axis=1) * A_SCALE).astype(f8)),
        })
    r0 = run("A0", in_0, list(range(8)))
    Mfull = C(np.concatenate([r0[c]["Mc"] for c in range(8)], axis=0))
    V2 = [[C((r0[l * 4 + hd]["VT"].T * V_SCALE).astype(f8)) for hd in range(4)] for l in range(2)]

    # ---- launch P1: edge build ----
    in_a = []
    for c in range(8):
        b, rcn = c // 4, c % 4
        in_a.append({
            "xT": xTb[b], "xTc": C(xTb[b][:, rcn * CH:(rcn + 1) * CH]),
            "Mm": Mfull,
            "srcx": C(np.arange(rcn * CH, (rcn + 1) * CH, dtype=np.float32)[:, None]),
        })
    ra = run("A", in_a, list(range(8)))
    topi = np.stack([np.concatenate([ra[b * 4 + r]["topi"] for r in range(4)], 0) for b in range(B)])
    ew = np.stack([np.concatenate([ra[b * 4 + r]["ew"] for r in range(4)], 0) for b in range(B)])
    tpi16 = [C(topi[b].astype(np.int16)) for b in range(B)]
    ew16 = [C(ew[b].astype(bf16)) for b in range(B)]

    # ---- launches P2, P3: the two GAT layers (host pre-sums partials) ----
    sel2 = C(np.stack([np.zeros(128, f32), np.ones(128, f32)]).astype(bf16))
    xin8 = xTb8
    for li, tag in enumerate(("B", "C")):
        in_l = []
        for c in range(8):
            b, hd = c // 4, c % 4
            w32 = glw[li, hd * H:(hd + 1) * H, :].T * W_SCALE
            wh = w32.astype(f8)
            wl = (w32 - wh.astype(f32)).astype(f8)
            in_l.append({
                "xT": xin8[b],
                "WTh": C(wh), "WTl": C(wl),
                "V2": V2[li][hd],
                "tpi": tpi16[b], "ewd": ew16[b],
                "sl2": sel2,
            })
        rl = run(tag, in_l, list(range(8)))
        xin8, xacc = [], []
        for b in range(B):
            acc = sum(rl[b * 4 + i]["gT"].astype(f32) / (rl[b * 4 + i]["atO"][0] + 1e-8)
                      for i in range(4)) / (HEADS * W_SCALE)
            acc = np.maximum(acc, 0.0)
            xacc.append(acc)
            xin8.append(C(acc.astype(f8)))

    # ---- launch P4: pooling + projection head ----
    in_d = []
    for b in range(B):
        in_d.append({
            "x2T": C(xacc[b].astype(bf16)),
            "x2n": C(xacc[b].T.astype(bf16)),
            "wpc": C(np.asarray(wp, f32).reshape(H, 1).astype(bf16)),
            "w1T": C(np.asarray(w1, f32).T.astype(bf16)),
            "b1c": C(np.asarray(b1, f32)[:, None]),
            "w2T": C(np.asarray(w2, f32).T.astype(bf16)),
            "b2c": C(np.asarray(b2, f32)[:, None]),
        })
    rd = run("D", in_d, [0, 1])
    out = np.stack([rd[b]["res"][:, 0].astype(f32) for b in range(B)])
    if _profile is not None:
        _profile.update(times)
    return out


# revision 60
# speedup vs baseline: 1.0256x; 1.0236x over previous
"""Trainium2 Bass kernel for nn_GraphSemanticExtractor (GNN message passing).

Sharding (8 NeuronCores), 5 launches with host-side layout glue between them:
  P0: core c => 128-row chunk of M = phi_w @ psi_w.T, plus V = W^T [a_src|a_dst]
      for (layer l=c//4, head hd=c%4).
  P1 (edge build): core c => (batch b=c//4, 256-row chunk rc=c%4);
      scores = (x_c @ M) @ x.T, top-8, softmax over the 8, self-edge mask.
  P2/P3 (GAT layers 1/2): core c => (batch b=c//4, head hd=c%4); between the
      two, the host computes x1 = relu(sum of per-head partials).
  P4: pool + projection head, core b in {0,1}.

The sparse top-k aggregation out[dst] += wgt*h[src] is a dense matmul
out.T = h.T @ R with R[s,t] = ew_k(s)*exp(lrelu(e_src[s]+e_dst[t])) at
t=topi[s,k]; R is built by scattering ew into M0 (gpsimd local_scatter) and a
dense lrelu/exp of the rank-1 e-grid, all overlapped with the h matmul on PE.
"""

import sys

sys.path.insert(0, "/opt/trn_rl_repo")
sys.path.insert(0, "/opt/trn_rl_repo/concourse")

from contextlib import ExitStack

import ml_dtypes
import numpy as np

import concourse.bass as bass
import concourse.tile as tile
from concourse import bacc, mybir
from concourse.bass_utils import run_bass_kernel_spmd

F32 = mybir.dt.float32
BF16 = mybir.dt.bfloat16
F8 = mybir.dt.float8e4
U32 = mybir.dt.uint32
I16 = mybir.dt.int16
AF = mybir.ActivationFunctionType
OP = mybir.AluOpType
AX = mybir.AxisListType
DR = mybir.MatmulPerfMode.DoubleRow

B, S, H = 2, 1024, 1024
HEADS, K = 4, 8
SEM = 512
NB = H // 128  # 8 partition blocks
ND = NB // 2   # 4 double-row blocks for fp8 DoubleRow matmuls
CH = S // 4    # 256 rows per edge-build core

# fp8 e4m3 has min-normal 2^-6; the tiny GAT weights (~0.02 scale) are scaled
# up on the host and the factors folded back into on-device scalars.
W_SCALE = 32.0   # W.T fed to the h matmul
V_SCALE = 64.0   # V = W^T [a_src|a_dst] fed to the e matmuls
A_SCALE = 64.0   # gat_att halves fed to P0's V matmul


def _build_P0(nc):
    """Per core: 128 rows of M = phi_w.T @ psi_w, and V = W^T [a_src|a_dst]
    for one (layer, head).  The V matmul runs in fp8 DoubleRow (host scales
    its inputs by A_SCALE*W_SCALE; the eviction scales back)."""
    pTc = nc.dram_tensor("pTc", [H, 128], BF16, kind="ExternalInput")
    sT = nc.dram_tensor("sT", [H, H], BF16, kind="ExternalInput")
    Wn = nc.dram_tensor("Wn", [H, H], F8, kind="ExternalInput")
    a2 = nc.dram_tensor("a2", [H, 2], F8, kind="ExternalInput")
    Mc = nc.dram_tensor("Mc", [128, H], BF16, kind="ExternalOutput")
    VT = nc.dram_tensor("VT", [2, H], F32, kind="ExternalOutput")

    with tile.TileContext(nc) as tc, ExitStack() as ctx:
        pers = ctx.enter_context(tc.tile_pool(name="pers", bufs=1))
        psum = ctx.enter_context(tc.tile_pool(name="psum", bufs=4, space="PSUM"))

        # all input DMAs on the sync queue, smallest first (single DMA engine
        # processes FIFO; a big transfer queued first would stall the rest)
        a2t = pers.tile([128, NB, 2], F8, tag="a2t")
        nc.sync.dma_start(out=a2t[:], in_=a2[:].rearrange("(kb p) c -> p kb c", p=128))
        pT16 = pers.tile([128, NB, 128], BF16, tag="pT16")
        nc.sync.dma_start(out=pT16[:], in_=pTc[:].rearrange("(kb p) c -> p kb c", p=128))
        Wn16 = pers.tile([128, NB, H], F8, tag="Wn16")
        nc.sync.dma_start(out=Wn16[:], in_=Wn[:].rearrange("(kb p) s -> p kb s", p=128))
        sT16 = pers.tile([128, NB, H], BF16, tag="sT16")
        nc.sync.dma_start(out=sT16[:], in_=sT[:].rearrange("(kb p) s -> p kb s", p=128))

        Vt = pers.tile([2, H], F32, tag="Vt")
        for n0 in range(0, H, 512):
            pt = psum.tile([2, 512], F32, tag="vm")
            for k in range(NB):
                nc.tensor.matmul(pt[:], a2t[:, k, :], Wn16[:, k, n0:n0 + 512],
                                 start=(k == 0), stop=(k == NB - 1))
            nc.vector.tensor_scalar(Vt[:, n0:n0 + 512], pt[:],
                                    1.0 / (A_SCALE * W_SCALE), None, op0=OP.mult)
        nc.sync.dma_start(out=VT[:], in_=Vt[:])

        Mc16 = pers.tile([128, H], BF16, tag="Mc16")
        for n0 in range(0, H, 512):
            pt = psum.tile([128, 512], F32, tag="mm")
            for k in range(NB):
                nc.tensor.matmul(pt[:], pT16[:, k, :], sT16[:, k, n0:n0 + 512],
                                 start=(k == 0), stop=(k == NB - 1))
            nc.vector.tensor_copy(out=Mc16[:, n0:n0 + 512], in_=pt[:])
        nc.sync.dma_start(out=Mc[:], in_=Mc16[:])
    nc.compile()
    return nc


def _build_P1(nc):
    """Edge build: scores = (x_c @ M) @ x.T, top-8 + softmax + self-mask."""
    xT = nc.dram_tensor("xT", [H, S], BF16, kind="ExternalInput")
    xTc = nc.dram_tensor("xTc", [H, CH], BF16, kind="ExternalInput")
    Mm = nc.dram_tensor("Mm", [H, H], BF16, kind="ExternalInput")
    srcx = nc.dram_tensor("srcx", [CH, 1], F32, kind="ExternalInput")
    Vb = nc.dram_tensor("Vb", [H, 2], BF16, kind="ExternalInput")
    topi = nc.dram_tensor("topi", [CH, K], U32, kind="ExternalOutput")
    ew = nc.dram_tensor("ew", [CH, K], F32, kind="ExternalOutput")
    ebO = nc.dram_tensor("ebO", [2, S], BF16, kind="ExternalOutput")

    with tile.TileContext(nc) as tc, ExitStack() as ctx:
        pers = ctx.enter_context(tc.tile_pool(name="pers", bufs=1))
        psum = ctx.enter_context(tc.tile_pool(name="psum", bufs=4, space="PSUM"))
        psumb = ctx.enter_context(tc.tile_pool(name="psumb", bufs=4, space="PSUM"))

        sx = pers.tile([128, 2, 1], F32, tag="sx")
        nc.sync.dma_start(out=sx[:], in_=srcx[:].rearrange("(m p) c -> p m c", p=128))
        Vb16 = pers.tile([128, NB, 2], BF16, tag="Vb16")
        nc.sync.dma_start(out=Vb16[:], in_=Vb[:].rearrange("(kb p) c -> p kb c", p=128))
        xTc16 = pers.tile([128, NB, CH], BF16, tag="xTc16")
        nc.sync.dma_start(out=xTc16[:], in_=xTc[:].rearrange("(kb p) s -> p kb s", p=128))
        M16 = pers.tile([128, NB, H], BF16, tag="M16")
        nc.sync.dma_start(out=M16[:], in_=Mm[:].rearrange("(kb p) s -> p kb s", p=128))
        xT16 = pers.tile([128, NB, S], BF16, tag="xT16")
        nc.sync.dma_start(out=xT16[:], in_=xT[:].rearrange("(kb p) s -> p kb s", p=128))

        # preload the Exp act table while DMAs run so the top-k chain's exp
        # doesn't pay the 1.3us table load; warm the PE p-state too
        warm = pers.tile([1, 1], F32, tag="warm")
        nc.vector.memset(warm[:], 0.0)
        nc.scalar.activation(warm[:], warm[:], AF.Exp)

        # PT[j, s-chunk] = (x_c @ M).T  (j = feature of M's column space)
        PT16 = pers.tile([128, NB, CH], BF16, tag="PT16")
        for m in range(NB):
            pt = psumb.tile([128, CH], F32, tag="ptm")
            for k in range(NB):
                nc.tensor.matmul(pt[:], M16[:, k, m * 128:(m + 1) * 128], xTc16[:, k, :],
                                 start=(k == 0), stop=(k == NB - 1))
            nc.vector.tensor_copy(out=PT16[:, m, :], in_=pt[:])

        # scores [s-chunk, t] f32; the whole per-sb top-8/softmax/mask chain runs
        # while the other sb-block's matmuls occupy PE
        sc = pers.tile([128, 2, S], F32, tag="scores")
        mv = pers.tile([128, 2, K], F32, tag="mv")
        ti = pers.tile([128, 2, K], U32, tag="ti")
        ex = pers.tile([128, 2, K], F32, tag="ex")
        sm = pers.tile([128, 2, 1], F32, tag="sm")
        rc = pers.tile([128, 2, 1], F32, tag="rc")
        tif = pers.tile([128, 2, K], F32, tag="tif")
        w8 = pers.tile([128, 2, K], F32, tag="w8")
        msk = pers.tile([128, 2, K], F32, tag="msk")
        ewt = pers.tile([128, 2, K], F32, tag="ewt")
        topir = topi[:].rearrange("(m p) k -> p m k", p=128)
        ewr = ew[:].rearrange("(m p) k -> p m k", p=128)
        for sb in range(2):
            for n0 in range(0, S, 512):
                pt = psum.tile([128, 512], F32, tag="scm")
                for k in range(NB):
                    nc.tensor.matmul(pt[:], PT16[:, k, sb * 128:(sb + 1) * 128],
                                     xT16[:, k, n0:n0 + 512],
                                     start=(k == 0), stop=(k == NB - 1))
                nc.vector.tensor_copy(out=sc[:, sb, n0:n0 + 512], in_=pt[:])
            nc.vector.max(mv[:, sb, :], sc[:, sb, :])
            nc.vector.max_index(ti[:, sb, :], mv[:, sb, :], sc[:, sb, :])
            nc.sync.dma_start(out=topir[:, sb, :], in_=ti[:, sb, :])
            nc.scalar.activation(ex[:, sb, :], mv[:, sb, :], AF.Exp)
            nc.vector.tensor_reduce(sm[:, sb, :], ex[:, sb, :], axis=AX.X, op=OP.add)
            nc.vector.tensor_scalar(sm[:, sb, :], sm[:, sb, :], 1e-8, None, op0=OP.add)
            nc.vector.reciprocal(rc[:, sb, :], sm[:, sb, :])
            nc.vector.tensor_copy(out=tif[:, sb, :], in_=ti[:, sb, :])
            nc.vector.tensor_scalar(w8[:, sb, :], ex[:, sb, :], rc[:, sb, :], 1e-8, op0=OP.mult, op1=OP.max)
            nc.vector.tensor_scalar(msk[:, sb, :], tif[:, sb, :], sx[:, sb, :], None, op0=OP.is_equal)
            nc.vector.tensor_scalar(msk[:, sb, :], msk[:, sb, :], -1.0, 1.0, op0=OP.mult, op1=OP.add)
            nc.vector.tensor_tensor(ewt[:, sb, :], w8[:, sb, :], msk[:, sb, :], op=OP.mult)
            nc.sync.dma_start(out=ewr[:, sb, :], in_=ewt[:, sb, :])

        # layer-0 e-values for head hd=rc on the now-hot PE (feeds launch B)
        ebo = pers.tile([2, S], BF16, tag="ebo")
        for n0 in range(0, S, 512):
            pt = psum.tile([2, 512], F32, tag="scm")
            for k in range(NB):
                nc.tensor.matmul(pt[:], Vb16[:, k, :], xT16[:, k, n0:n0 + 512],
                                 start=(k == 0), stop=(k == NB - 1))
            nc.vector.tensor_copy(out=ebo[:, n0:n0 + 512], in_=pt[:])
        nc.sync.dma_start(out=ebO[:], in_=ebo[:])
    nc.compile()
    return nc


def _build_L(nc, eb_in=False):
    """One GAT layer for one (batch, head).  gT[feat, node] = (agg/attn)/HEADS.
    The h matmul runs fp8 DoubleRow with W split into an fp8 hi/lo residual
    pair (hi + lo/16 restores ~bf16 weight accuracy; fp8 W alone costs 2e-2
    output error).  x and V are plain fp8 (~2e-3 each).  R and the
    aggregation stay bf16: fp8 R alone costs 4e-2."""
    xT = nc.dram_tensor("xT", [H, S], F8, kind="ExternalInput")
    WTh = nc.dram_tensor("WTh", [H, H], F8, kind="ExternalInput")
    WTl = nc.dram_tensor("WTl", [H, H], F8, kind="ExternalInput")
    if eb_in:
        ebI = nc.dram_tensor("ebI", [2, S], BF16, kind="ExternalInput")
    else:
        V2 = nc.dram_tensor("V2", [H, 2], F8, kind="ExternalInput")
    tpi = nc.dram_tensor("tpi", [S, K], I16, kind="ExternalInput")
    ewd = nc.dram_tensor("ewd", [S, K], BF16, kind="ExternalInput")
    sl2 = nc.dram_tensor("sl2", [2, 128], BF16, kind="ExternalInput")
    gT = nc.dram_tensor("gT", [H, S], BF16, kind="ExternalOutput")
    atO = nc.dram_tensor("atO", [1, S], F32, kind="ExternalOutput")

    with tile.TileContext(nc) as tc, ExitStack() as ctx:
        pers = ctx.enter_context(tc.tile_pool(name="pers", bufs=1))
        psum = ctx.enter_context(tc.tile_pool(name="psum", bufs=2, space="PSUM"))
        pse = ctx.enter_context(tc.tile_pool(name="pse", bufs=1, space="PSUM"))
        psx = ctx.enter_context(tc.tile_pool(name="psx", bufs=1, space="PSUM"))

        # all inputs on the sync queue, smallest first (FIFO DMA engine)
        sel = pers.tile([2, 128], BF16, tag="sel")
        nc.sync.dma_start(out=sel[:], in_=sl2[:])
        if eb_in:
            ebL = pers.tile([2, S], BF16, tag="ebL")
            nc.sync.dma_start(out=ebL[:], in_=ebI[:])
        else:
            V16 = pers.tile([128, NB, 2], F8, tag="V16")
            nc.sync.dma_start(out=V16[:], in_=V2[:].rearrange("(kb p) c -> p kb c", p=128))
        tpw = pers.tile([128, NB, K], I16, tag="tpw")
        nc.sync.dma_start(out=tpw[:], in_=tpi[:].rearrange("(m p) k -> p m k", p=128))
        ews16 = pers.tile([128, NB, K], BF16, tag="ews16")
        nc.sync.dma_start(out=ews16[:], in_=ewd[:].rearrange("(m p) k -> p m k", p=128))
        xT16 = pers.tile([128, NB, S], F8, tag="xT16")  # noqa: dma order: tiny first
        nc.sync.dma_start(out=xT16[:], in_=xT[:].rearrange("(kb p) s -> p kb s", p=128))
        WTh16 = pers.tile([128, NB, H], F8, tag="WTh16")
        nc.sync.dma_start(out=WTh16[:], in_=WTh[:].rearrange("(kb p) s -> p kb s", p=128))
        WTl16 = pers.tile([128, NB, H], F8, tag="WTl16")
        nc.sync.dma_start(out=WTl16[:], in_=WTl[:].rearrange("(kb p) s -> p kb s", p=128))

        ones11 = pers.tile([1, 1], BF16, tag="ones11")
        nc.vector.memset(ones11[:], 1.0)
        # preload the Lrelu act table while DMAs run
        warm = pers.tile([1, 1], F32, tag="warm")
        nc.vector.memset(warm[:], 0.0)
        nc.scalar.activation(warm[:], warm[:], AF.Lrelu, alpha=0.2)

        # gpsimd: M0 blocks (scatter of ew into dense [s, t])
        M0 = pers.tile([128, NB, S], BF16, tag="M0")
        for m in range(NB):
            nc.gpsimd.local_scatter(M0[:, m, :], ews16[:, m, :], tpw[:, m, :],
                                    channels=128, num_elems=S, num_idxs=K)

        # PE: e_bothT [2, node] = V^T x as two independent k-split psum groups
        # (independent groups queue up and run at ramped PE speed); eb2 keeps
        # the V_SCALE factor — the exp pass divides it out via its scale param
        # (lrelu is positively homogeneous so the order is exact)
        eb2 = pers.tile([2, S], BF16, tag="eb2")
        tmpe = pers.tile([2, 2, 512], BF16, tag="tmpe")
        edb = pers.tile([128, S], F32, tag="edb")
        esc = pers.tile([128, NB, 1], F32, tag="esc")
        for i, n0 in enumerate(range(0, S, 512)):
            if eb_in:
                nc.vector.tensor_copy(out=eb2[:, n0:n0 + 512], in_=ebL[:, n0:n0 + 512])
            else:
                ptA = pse.tile([2, 512], F32, tag="ebp")
                ptB = pse.tile([2, 512], F32, tag="ebp2")
                for kk in range(4):
                    for j, pt in ((0, ptA), (4, ptB)):
                        nc.tensor.matmul(pt[:], V16[:, j + kk, :], xT16[:, j + kk, n0:n0 + 512],
                                         start=(kk == 0), stop=(kk == 3))
                nc.vector.tensor_copy(out=tmpe[:, i, :], in_=ptA[:])
                nc.vector.tensor_tensor(eb2[:, n0:n0 + 512], tmpe[:, i, :], ptB[:], op=OP.add)
            # this half's e_dst broadcast and e_src transposes, so the Act
            # lrelu chain can start before the other half's matmuls finish
            ptd = psx.tile([128, 512], F32, tag="edbp")
            nc.tensor.matmul(ptd[:], sel[:], eb2[:, n0:n0 + 512], start=True, stop=True)
            nc.scalar.copy(out=edb[:, n0:n0 + 512], in_=ptd[:])
            for m in range(4 * i, 4 * i + 4):
                pt = psx.tile([128, 1], F32, tag="escp")
                nc.tensor.matmul(pt[:], eb2[0:1, m * 128:(m + 1) * 128], ones11[:],
                                 start=True, stop=True)
                nc.vector.tensor_copy(out=esc[:, m, :], in_=pt[:])

        # PE: h [node, feat] bf16, W_SCALE-scaled.  Two fp8 DoubleRow passes
        # (W-hi, then the natural-scale fp8 residual W-lo, which lands in the
        # denormal range) accumulate into ONE psum group — bf16-level weight
        # accuracy at fp8-DR speed, plain copy eviction.
        h16 = pers.tile([128, NB, H], BF16, tag="h16")
        for m in range(NB):
            for n0 in range(0, H, 512):
                pt = psum.tile([128, 512], F32, tag="mmp")
                for i, wt in enumerate((WTh16, WTl16)):
                    for d in range(ND):
                        nc.tensor.matmul(pt[:], xT16[:, 2 * d:2 * d + 2, m * 128:(m + 1) * 128],
                                         wt[:, 2 * d:2 * d + 2, n0:n0 + 512],
                                         start=(i == 0 and d == 0),
                                         stop=(i == 1 and d == ND - 1), perf_mode=DR)
                nc.vector.tensor_copy(out=h16[:, m, n0:n0 + 512], in_=pt[:])

        # Act: lrelu in column halves (starts as soon as the first edb half and
        # the matching esc blocks exist), then all exp; table loads: preloaded
        # Lrelu + one swap to Exp
        zl8 = pers.tile([128, NB, S], BF16, tag="zl8")
        for m in range(4):
            nc.scalar.activation(zl8[:, m, 0:512], edb[:, 0:512], AF.Lrelu,
                                 bias=esc[:, m, :], alpha=0.2)
        for m in range(4):
            nc.scalar.activation(zl8[:, m, 512:1024], edb[:, 512:1024], AF.Lrelu,
                                 bias=esc[:, m, :], alpha=0.2)
        for m in range(4, NB):
            nc.scalar.activation(zl8[:, m, 0:512], edb[:, 0:512], AF.Lrelu,
                                 bias=esc[:, m, :], alpha=0.2)
        for m in range(4, NB):
            nc.scalar.activation(zl8[:, m, 512:1024], edb[:, 512:1024], AF.Lrelu,
                                 bias=esc[:, m, :], alpha=0.2)
        ez8 = pers.tile([128, NB, S], BF16, tag="ez8")
        for m in range(NB):
            nc.scalar.activation(ez8[:, m, :], zl8[:, m, :], AF.Exp, scale=1.0 / V_SCALE)
        R = pers.tile([128, NB, S], BF16, tag="R")
        for m in range(2):
            nc.gpsimd.tensor_tensor(R[:, m, :], M0[:, m, :], ez8[:, m, :], op=OP.mult)
        for m in range(2, NB):
            nc.vector.tensor_tensor(R[:, m, :], M0[:, m, :], ez8[:, m, :], op=OP.mult)

        # PE: out^T [feat, t] = h^T R (raw; the host divides by attn); chunked DMA
        gsb = pers.tile([128, NB, S], BF16, tag="gsb")
        gTr = gT[:].rearrange("(m p) t -> p m t", p=128)
        for m in range(NB):
            for n0 in range(0, S, 512):
                pt = psum.tile([128, 512], F32, tag="mmp")
                for k in range(NB):
                    nc.tensor.matmul(pt[:], h16[:, k, m * 128:(m + 1) * 128],
                                     R[:, k, n0:n0 + 512],
                                     start=(k == 0), stop=(k == NB - 1))
                nc.vector.tensor_copy(out=gsb[:, m, n0:n0 + 512], in_=pt[:])
            nc.sync.dma_start(out=gTr[:, m, :], in_=gsb[:, m, :])

        # PE: attn^T [1, t] = 1^T R, after agg so it runs on a hot PE
        onesc = pers.tile([128, 1], BF16, tag="onesc")
        nc.vector.memset(onesc[:], 1.0)
        atT = pers.tile([1, S], F32, tag="atT")
        for n0 in range(0, S, 512):
            pt = pse.tile([1, 512], F32, tag="atp")
            for k in range(NB):
                nc.tensor.matmul(pt[:], onesc[:], R[:, k, n0:n0 + 512],
                                 start=(k == 0), stop=(k == NB - 1))
            nc.vector.tensor_copy(out=atT[:, n0:n0 + 512], in_=pt[:])
        nc.sync.dma_start(out=atO[:], in_=atT[:])
    nc.compile()
    return nc


def _build_D(nc):
    """Attention pool over nodes + 2-layer projection head, one batch per core."""
    x2T = nc.dram_tensor("x2T", [H, S], BF16, kind="ExternalInput")
    x2n = nc.dram_tensor("x2n", [S, H], BF16, kind="ExternalInput")
    wpc = nc.dram_tensor("wpc", [H, 1], BF16, kind="ExternalInput")
    w1T = nc.dram_tensor("w1T", [H, SEM], BF16, kind="ExternalInput")
    b1c = nc.dram_tensor("b1c", [SEM, 1], F32, kind="ExternalInput")
    w2T = nc.dram_tensor("w2T", [SEM, SEM], BF16, kind="ExternalInput")
    b2c = nc.dram_tensor("b2c", [SEM, 1], F32, kind="ExternalInput")
    res = nc.dram_tensor("res", [SEM, 1], F32, kind="ExternalOutput")

    with tile.TileContext(nc) as tc, ExitStack() as ctx:
        pers = ctx.enter_context(tc.tile_pool(name="pers", bufs=1))
        tmp = ctx.enter_context(tc.tile_pool(name="tmp", bufs=3))
        psum = ctx.enter_context(tc.tile_pool(name="psum", bufs=3, space="PSUM"))

        wp16 = pers.tile([128, NB, 1], BF16, tag="wp16")
        nc.sync.dma_start(out=wp16[:], in_=wpc[:].rearrange("(kb p) c -> p kb c", p=128))
        b1f = pers.tile([128, 4, 1], F32, tag="b1f")
        nc.sync.dma_start(out=b1f[:], in_=b1c[:].rearrange("(m p) c -> p m c", p=128))
        b2f = pers.tile([128, 4, 1], F32, tag="b2f")
        nc.sync.dma_start(out=b2f[:], in_=b2c[:].rearrange("(m p) c -> p m c", p=128))
        # x2T column-chunked so psc starts after the first quarter arrives
        x3T = pers.tile([128, NB, S], BF16, tag="x3T")
        x2Tr = x2T[:].rearrange("(kb p) s -> p kb s", p=128)
        for n0 in range(0, S, 256):
            nc.sync.dma_start(out=x3T[:, :, n0:n0 + 256], in_=x2Tr[:, :, n0:n0 + 256])
        x2t16 = pers.tile([128, NB, H], BF16, tag="x2t16")
        nc.sync.dma_start(out=x2t16[:], in_=x2n[:].rearrange("(tb p) f -> p tb f", p=128))
        w116 = pers.tile([128, NB, SEM], BF16, tag="w116")
        nc.sync.dma_start(out=w116[:], in_=w1T[:].rearrange("(kb p) c -> p kb c", p=128))
        w216 = pers.tile([128, 4, SEM], BF16, tag="w216")
        nc.sync.dma_start(out=w216[:], in_=w2T[:].rearrange("(kb p) c -> p kb c", p=128))

        # preload the Exp act table during the DMAs; warm the PE p-state
        warm = pers.tile([1, 1], F32, tag="warm")
        nc.vector.memset(warm[:], 0.0)
        nc.scalar.activation(warm[:], warm[:], AF.Exp)

        psc = pers.tile([1, S], F32, tag="psc")
        for n0 in range(0, S, 256):
            pt = psum.tile([1, 256], F32, tag="sp")
            for k in range(NB):
                nc.tensor.matmul(pt[:], wp16[:, k, :], x3T[:, k, n0:n0 + 256],
                                 start=(k == 0), stop=(k == NB - 1))
            nc.vector.tensor_copy(out=psc[:, n0:n0 + 256], in_=pt[:])

        mx = pers.tile([1, 1], F32, tag="mx")
        nc.vector.tensor_reduce(mx[:], psc[:], axis=AX.X, op=OP.max)
        nmx = pers.tile([1, 1], F32, tag="nmx")
        nc.vector.tensor_scalar(nmx[:], mx[:], -1.0, None, op0=OP.mult)
        ev = pers.tile([1, S], F32, tag="ev")
        nc.scalar.activation(ev[:], psc[:], AF.Exp, bias=nmx[:])
        sm = pers.tile([1, 1], F32, tag="sm")
        nc.vector.tensor_reduce(sm[:], ev[:], axis=AX.X, op=OP.add)
        rc = pers.tile([1, 1], F32, tag="rc")
        nc.vector.reciprocal(rc[:], sm[:])
        alT = pers.tile([1, S], BF16, tag="alT")
        nc.vector.tensor_scalar(alT[:], ev[:], rc[:], None, op0=OP.mult)

        # transpose alpha into partitions (8 tiny matmuls), then pooled = x2^T @ alpha
        # runs on PE instead of a serial DVE accumulation chain
        ones11 = pers.tile([1, 1], BF16, tag="ones11")
        nc.vector.memset(ones11[:], 1.0)
        alp = pers.tile([128, NB, 1], BF16, tag="alp")
        for tb in range(NB):
            pt = psum.tile([128, 1], F32, tag="sp1")
            nc.tensor.matmul(pt[:], alT[0:1, tb * 128:(tb + 1) * 128], ones11[:],
                             start=True, stop=True)
            nc.vector.tensor_copy(out=alp[:, tb, :], in_=pt[:])
        pld = pers.tile([128, NB, 1], BF16, tag="pld")
        for fb in range(NB):
            pt = psum.tile([128, 1], F32, tag="sp1")
            for tb in range(NB):
                nc.tensor.matmul(pt[:], x2t16[:, tb, fb * 128:(fb + 1) * 128], alp[:, tb, :],
                                 start=(tb == 0), stop=(tb == NB - 1))
            nc.vector.tensor_copy(out=pld[:, fb, :], in_=pt[:])

        hid = pers.tile([128, 4, 1], BF16, tag="hid")
        for m in range(4):
            pt = psum.tile([128, 1], F32, tag="sp1")
            for k in range(NB):
                nc.tensor.matmul(pt[:], w116[:, k, m * 128:(m + 1) * 128], pld[:, k, :],
                                 start=(k == 0), stop=(k == NB - 1))
            nc.scalar.activation(hid[:, m, :], pt[:], AF.Relu, bias=b1f[:, m, :])

        rsb = pers.tile([128, 4, 1], F32, tag="rsb")
        for m in range(4):
            pt = psum.tile([128, 1], F32, tag="sp1")
            for k in range(4):
                nc.tensor.matmul(pt[:], w216[:, k, m * 128:(m + 1) * 128], hid[:, k, :],
                                 start=(k == 0), stop=(k == 3))
            nc.vector.tensor_tensor(rsb[:, m, :], pt[:], b2f[:, m, :], op=OP.add)
        nc.sync.dma_start(out=res[:].rearrange("(m p) c -> p m c", p=128), in_=rsb[:])
    nc.compile()
    return nc


_PROGS = {}


def _get_progs():
    if not _PROGS:
        def mk():
            return bacc.Bacc("TRN2", target_bir_lowering=False, debug=False,
                             enable_asserts=True, num_devices=8)
        _PROGS["A0"] = _build_P0(mk())
        _PROGS["A"] = _build_P1(mk())
        _PROGS["B"] = _build_L(mk(), eb_in=True)
        _PROGS["C"] = _build_L(mk())
        _PROGS["D"] = _build_D(mk())
    return _PROGS


def kernel(hidden_states, phi_w, psi_w, gat_lin_w, gat_att, wp, w1, b1, w2, b2,
           _profile=None):
    f32 = np.float32
    bf16 = ml_dtypes.bfloat16
    hidden_states = np.asarray(hidden_states, f32)
    progs = _get_progs()
    C = lambda a: np.ascontiguousarray(a)
    times = {}

    def run(tag, in_maps, core_ids):
        r = run_bass_kernel_spmd(progs[tag], in_maps, core_ids=core_ids)
        if _profile is not None:
            times[tag] = r.exec_time_ns
        return r.results

    f8 = ml_dtypes.float8_e4m3
    glw = np.asarray(gat_lin_w, f32)
    ga = np.asarray(gat_att, f32)
    xTb = [C(hidden_states[b].T.astype(bf16)) for b in range(B)]
    xTb8 = [C(hidden_states[b].T.astype(f8)) for b in range(B)]

    # ---- launch P0: M = phi_w.T @ psi_w chunks, V = W^T [a_src|a_dst] ----
    # (reference einsum 'bsd,ed->bse' is x @ phi_w.T, so scores = x M x.T with
    # M = phi_w.T @ psi_w; the contraction runs over the e rows of both.)
    pT = np.asarray(phi_w, f32).astype(bf16)
    sT = C(np.asarray(psi_w, f32).astype(bf16))
    in_0 = []
    for c in range(8):
        l, hd = c // 4, c % 4
        in_0.append({
            "pTc": C(pT[:, c * 128:(c + 1) * 128]),
            "sT": sT,
            "Wn": C((glw[l, hd * H:(hd + 1) * H, :] * W_SCALE).astype(f8)),
            "a2": C((np.stack([ga[l, hd, :H], ga[l, hd, H:]], axis=1) * A_SCALE).astype(f8)),
        })
    r0 = run("A0", in_0, list(range(8)))
    Mfull = C(np.concatenate([r0[c]["Mc"] for c in range(8)], axis=0))
    V2 = [[C((r0[l * 4 + hd]["VT"].T * V_SCALE).astype(f8)) for hd in range(4)] for l in range(2)]

    # ---- launch P1: edge build ----
    in_a = []
    for c in range(8):
        b, rcn = c // 4, c % 4
        in_a.append({
            "xT": xTb[b], "xTc": C(xTb[b][:, rcn * CH:(rcn + 1) * CH]),
            "Mm": Mfull,
            "srcx": C(np.arange(rcn * CH, (rcn + 1) * CH, dtype=np.float32)[:, None]),
            "Vb": C((r0[rcn]["VT"].T * V_SCALE).astype(bf16)),
        })
    ra = run("A", in_a, list(range(8)))
    topi = np.stack([np.concatenate([ra[b * 4 + r]["topi"] for r in range(4)], 0) for b in range(B)])
    ew = np.stack([np.concatenate([ra[b * 4 + r]["ew"] for r in range(4)], 0) for b in range(B)])
    tpi16 = [C(topi[b].astype(np.int16)) for b in range(B)]
    ew16 = [C(ew[b].astype(bf16)) for b in range(B)]

    # ---- launches P2, P3: the two GAT layers (host pre-sums partials) ----
    sel2 = C(np.stack([np.zeros(128, f32), np.ones(128, f32)]).astype(bf16))
    xin8 = xTb8
    for li, tag in enumerate(("B", "C")):
        in_l = []
        for c in range(8):
            b, hd = c // 4, c % 4
            w32 = glw[li, hd * H:(hd + 1) * H, :].T * W_SCALE
            wh = w32.astype(f8)
            wl = (w32 - wh.astype(f32)).astype(f8)
            d = {
                "xT": xin8[b],
                "WTh": C(wh), "WTl": C(wl),
                "tpi": tpi16[b], "ewd": ew16[b],
                "sl2": sel2,
            }
            if li == 0:
                d["ebI"] = C(np.asarray(ra[b * 4 + hd]["ebO"], bf16))
            else:
                d["V2"] = V2[li][hd]
            in_l.append(d)
        rl = run(tag, in_l, list(range(8)))
        xin8, xacc = [], []
        for b in range(B):
            acc = sum(rl[b * 4 + i]["gT"].astype(f32) / (rl[b * 4 + i]["atO"][0] + 1e-8)
                      for i in range(4)) / (HEADS * W_SCALE)
            acc = np.maximum(acc, 0.0)
            xacc.append(acc)
            xin8.append(C(acc.astype(f8)))

    # ---- launch P4: pooling + projection head ----
    in_d = []
    for b in range(B):
        in_d.append({
            "x2T": C(xacc[b].astype(bf16)),
            "x2n": C(xacc[b].T.astype(bf16)),
            "wpc": C(np.asarray(wp, f32).reshape(H, 1).astype(bf16)),
            "w1T": C(np.asarray(w1, f32).T.astype(bf16)),
            "b1c": C(np.asarray(b1, f32)[:, None]),
            "w2T": C(np.asarray(w2, f32).T.astype(bf16)),
            "b2c": C(np.asarray(b2, f32)[:, None]),
        })
    rd = run("D", in_d, [0, 1])
    out = np.stack([rd[b]["res"][:, 0].astype(f32) for b in range(B)])
    if _profile is not None:
        _profile.update(times)
    return out
